# Initial kernel scaffold
#
"""GQA attention block (RMSNorm-QK + RoPE + causal attention + proj) on 8 TRN2 cores.

Sharding: DP=2 over batch x TP=4 over heads (4 q heads + 1 kv head per core).
Per core: x[b] @ Wq_shard / Wkv_shard -> q,k,v; RMSNorm+RoPE (cos/sin tables
precomputed on host, q_scale/k_scale and 1/sqrt(HS) baked in); causal
flash-ish attention in bf16 with f32 softmax stats; AllGather of y^T over the
4 TP ranks; column-parallel Wproj. Host pre-transposes x so the device never
transposes activations for the projections.
"""

import math
import os
import sys

import numpy as np

for _p in ("/opt/trn_rl_repo", "/root/.axon_site/_ro/trn_rl_repo"):
    if os.path.isdir(_p) and _p not in sys.path:
        sys.path.insert(0, _p)

import ml_dtypes

import concourse.bacc as bacc
import concourse.mybir as mybir
import concourse.tile as tile
from concourse import masks
from concourse.bass_utils import run_bass_kernel_spmd

BF16 = mybir.dt.bfloat16
F32 = mybir.dt.float32
AX = mybir.AxisListType
ALU = mybir.AluOpType
AF = mybir.ActivationFunctionType

B, T, C = 2, 2048, 2048
NH, NKV, HS = 16, 4, 128
TP = 4                # tensor-parallel ranks per batch element
QH = NH // TP         # q heads per core
QW = QH * HS          # 512
PT = 128
NT = T // PT          # 16
NCT = C // PT         # 16
H2 = HS // 2
EPS = 1e-6
THETA = 10000.0
NCORES = 8
BF = ml_dtypes.bfloat16

_CACHE = {}


def _build():
    nc = bacc.Bacc(None, target_bir_lowering=False, num_devices=NCORES)

    xT = nc.declare_dram_parameter("xT", [C, T], BF16, isOutput=False)
    wq = nc.declare_dram_parameter("wq", [C, QW], BF16, isOutput=False)
    wkv = nc.declare_dram_parameter("wkv", [C, 2 * HS], BF16, isOutput=False)
    wp = nc.declare_dram_parameter("wp", [C, QW], BF16, isOutput=False)
    v1s = nc.declare_dram_parameter("v1s", [T, HS], F32, isOutput=False)
    cosq = nc.declare_dram_parameter("cosq", [T, QW], BF16, isOutput=False)
    sinq = nc.declare_dram_parameter("sinq", [T, QW], BF16, isOutput=False)
    cosk = nc.declare_dram_parameter("cosk", [T, HS], BF16, isOutput=False)
    sink = nc.declare_dram_parameter("sink", [T, HS], BF16, isOutput=False)
    mneg = nc.declare_dram_parameter("mneg", [PT, PT], F32, isOutput=False)
    out = nc.declare_dram_parameter("out", [T, QW], F32, isOutput=True)

    groups = [[0, 1, 2, 3], [4, 5, 6, 7]]

    with tile.TileContext(nc) as tc:
        with (
            tc.tile_pool(name="const", bufs=1) as const,
            tc.tile_pool(name="persist", bufs=1) as persist,
            tc.tile_pool(name="psum", bufs=1, space="PSUM") as psum,
            tc.tile_pool(name="wk", bufs=3) as wk,
            tc.tile_pool(name="dram", bufs=1, space="DRAM") as dram,
        ):
            ident = const.tile([PT, PT], BF16)
            masks.make_identity(nc, ident[:])
            maskt = const.tile([PT, PT], F32)
            nc.sync.dma_start(maskt[:], mneg[:])

            wq_s = persist.tile([PT, NCT, QW], BF16)
            nc.sync.dma_start(wq_s[:], wq[:].rearrange("(c p) m -> p c m", p=PT))
            wkv_s = persist.tile([PT, NCT, 2 * HS], BF16)
            nc.sync.dma_start(wkv_s[:], wkv[:].rearrange("(c p) m -> p c m", p=PT))
            wp_s = persist.tile([PT, NCT, QW], BF16)
            nc.sync.dma_start(wp_s[:], wp[:].rearrange("(c p) m -> p c m", p=PT))

            qT_s = persist.tile([PT, QH, T], BF16)
            kT_s = persist.tile([PT, T], BF16)
            v_s = persist.tile([PT, NT, HS], BF16)
            yT_s = persist.tile([PT, QH, T], BF16)

            ag_in = dram.tile([QW, T], BF16)
            ag_out = dram.tile([C, T], BF16, addr_space="Shared")

            with tc.tile_pool(name="xtp", bufs=1) as xtp:
                xt_s = xtp.tile([PT, NCT, T], BF16)
                nc.sync.dma_start(xt_s[:], xT[:].rearrange("(c p) t -> p c t", p=PT))

                for ti in range(NT):
                    t0 = ti * PT
                    # ---- QKV projection (lhsT = xT tile, rhs = weight) ----
                    qp = psum.tile([PT, QW], F32, tag="qp", bufs=1)
                    for ci in range(NCT):
                        nc.tensor.matmul(
                            qp[:], xt_s[:, ci, t0:t0 + PT], wq_s[:, ci, :],
                            start=(ci == 0), stop=(ci == NCT - 1),
                        )
                    kvp = psum.tile([PT, 2 * HS], F32, tag="kvp", bufs=1)
                    for ci in range(NCT):
                        nc.tensor.matmul(
                            kvp[:], xt_s[:, ci, t0:t0 + PT], wkv_s[:, ci, :],
                            start=(ci == 0), stop=(ci == NCT - 1),
                        )

                    # ---- RMSNorm stats ----
                    sq = wk.tile([PT, QW], F32, tag="sq")
                    nc.scalar.square(sq[:], qp[:])
                    ms = wk.tile([PT, QH], F32, tag="ms")
                    nc.vector.tensor_reduce(
                        ms[:], sq[:].rearrange("p (h d) -> p h d", d=HS), AX.X, ALU.add
                    )
                    sqk = wk.tile([PT, HS], F32, tag="sqk")
                    nc.scalar.square(sqk[:], kvp[:, 0:HS])
                    msk = wk.tile([PT, 1], F32, tag="msk")
                    nc.vector.tensor_reduce(msk[:], sqk[:], AX.X, ALU.add)

                    rs = wk.tile([PT, QH], F32, tag="rs")
                    nc.scalar.activation(rs[:], ms[:], AF.Sqrt, bias=EPS, scale=1.0 / HS)
                    nc.vector.reciprocal(rs[:], rs[:])
                    rsk = wk.tile([PT, 1], F32, tag="rsk")
                    nc.scalar.activation(rsk[:], msk[:], AF.Sqrt, bias=EPS, scale=1.0 / HS)
                    nc.vector.reciprocal(rsk[:], rsk[:])

                    # ---- normalize (per-head scalar) ----
                    qn = wk.tile([PT, QH, HS], BF16, tag="qn")
                    for h in range(QH):
                        nc.vector.tensor_scalar_mul(
                            qn[:, h, :], qp[:, h * HS:(h + 1) * HS], rs[:, h:h + 1]
                        )
                    kn = wk.tile([PT, HS], BF16, tag="kn")
                    nc.vector.tensor_scalar_mul(kn[:], kvp[:, 0:HS], rsk[:])

                    # ---- RoPE: out = z*cos + rot(z)*sin, tables baked with scales ----
                    qrot = wk.tile([PT, QH, HS], BF16, tag="qrot")
                    nc.vector.tensor_scalar_mul(qrot[:, :, 0:H2], qn[:, :, H2:HS], -1.0)
                    nc.vector.tensor_copy(qrot[:, :, H2:HS], qn[:, :, 0:H2])
                    krot = wk.tile([PT, HS], BF16, tag="krot")
                    nc.vector.tensor_scalar_mul(krot[:, 0:H2], kn[:, H2:HS], -1.0)
                    nc.vector.tensor_copy(krot[:, H2:HS], kn[:, 0:H2])

                    cqt = wk.tile([PT, QW], BF16, tag="cqt")
                    nc.sync.dma_start(cqt[:], cosq[t0:t0 + PT, :])
                    sqt = wk.tile([PT, QW], BF16, tag="sqt")
                    nc.sync.dma_start(sqt[:], sinq[t0:t0 + PT, :])
                    ckt = wk.tile([PT, HS], BF16, tag="ckt")
                    nc.sync.dma_start(ckt[:], cosk[t0:t0 + PT, :])
                    skt = wk.tile([PT, HS], BF16, tag="skt")
                    nc.sync.dma_start(skt[:], sink[t0:t0 + PT, :])

                    qn2 = qn[:].rearrange("p h d -> p (h d)")
                    qrot2 = qrot[:].rearrange("p h d -> p (h d)")
                    qr = wk.tile([PT, QW], BF16, tag="qr")
                    nc.vector.tensor_tensor(qr[:], qn2, cqt[:], ALU.mult)
                    nc.vector.tensor_tensor(qrot2, qrot2, sqt[:], ALU.mult)
                    nc.vector.tensor_tensor(qr[:], qr[:], qrot2, ALU.add)

                    kr = wk.tile([PT, HS], BF16, tag="kr")
                    nc.vector.tensor_tensor(kr[:], kn[:], ckt[:], ALU.mult)
                    nc.vector.tensor_tensor(krot[:], krot[:], skt[:], ALU.mult)
                    nc.vector.tensor_tensor(kr[:], kr[:], krot[:], ALU.add)

                    # ---- v mix ----
                    v1t = wk.tile([PT, HS], F32, tag="v1t")
                    nc.sync.dma_start(v1t[:], v1s[t0:t0 + PT, :])
                    nc.vector.tensor_tensor(v_s[:, ti, :], kvp[:, HS:2 * HS], v1t[:], ALU.add)

                    # ---- transposes q,k -> qT, kT ----
                    qr3 = qr[:].rearrange("p (h d) -> p h d", d=HS)
                    for h in range(QH):
                        tq = psum.tile([PT, PT], F32, tag="tq", bufs=2)
                        nc.tensor.transpose(tq[:], qr3[:, h, :], ident[:])
                        nc.vector.tensor_copy(qT_s[:, h, t0:t0 + PT], tq[:])
                    tk = psum.tile([PT, PT], F32, tag="tq", bufs=2)
                    nc.tensor.transpose(tk[:], kr[:], ident[:])
                    nc.vector.tensor_copy(kT_s[:, t0:t0 + PT], tk[:])

                    # ---- causal attention row ti ----
                    nk = ti + 1
                    nchunk = (nk + 3) // 4
                    for h in range(QH):
                        prow = wk.tile([PT, T], BF16, tag="prow", bufs=2)
                        rsp = wk.tile([PT, 4], F32, tag="rsp", bufs=2)
                        for ch in range(nchunk):
                            c0 = ch * 512
                            cw = min(512, nk * PT - c0)
                            sp = psum.tile([PT, 512], F32, tag="sp", bufs=2)
                            nc.tensor.matmul(
                                sp[:, 0:cw], qT_s[:, h, t0:t0 + PT], kT_s[:, c0:c0 + cw],
                                start=True, stop=True,
                            )
                            if ch == nchunk - 1:
                                dc = cw - PT
                                nc.vector.tensor_tensor(
                                    sp[:, dc:dc + PT], sp[:, dc:dc + PT], maskt[:], ALU.add
                                )
                            nc.scalar.activation(
                                prow[:, c0:c0 + cw], sp[:, 0:cw], AF.Exp,
                                accum_out=rsp[:, ch:ch + 1],
                            )
                        rinv = wk.tile([PT, 1], F32, tag="rinv", bufs=2)
                        if nchunk > 1:
                            rsum = wk.tile([PT, 1], F32, tag="rsum", bufs=2)
                            nc.vector.tensor_reduce(rsum[:], rsp[:, 0:nchunk], AX.X, ALU.add)
                            nc.vector.reciprocal(rinv[:], rsum[:])
                        else:
                            nc.vector.reciprocal(rinv[:], rsp[:, 0:1])
                        nc.vector.tensor_scalar_mul(prow[:, 0:nk * PT], prow[:, 0:nk * PT], rinv[:])

                        yp = psum.tile([PT, HS], F32, tag="yp", bufs=2)
                        for j in range(nk):
                            ptp = psum.tile([PT, PT], F32, tag="tq", bufs=2)
                            nc.tensor.transpose(ptp[:], prow[:, j * PT:(j + 1) * PT], ident[:])
                            pts = wk.tile([PT, PT], BF16, tag="pts", bufs=3)
                            nc.vector.tensor_copy(pts[:], ptp[:])
                            nc.tensor.matmul(
                                yp[:], v_s[:, j, :], pts[:],
                                start=(j == 0), stop=(j == nk - 1),
                            )
                        nc.vector.tensor_copy(yT_s[:, h, t0:t0 + PT], yp[:])

            # ---- AllGather y^T over the TP group ----
            nc.sync.dma_start(ag_in[:].rearrange("(h p) t -> p h t", p=PT), yT_s[:])
            nc.gpsimd.collective_compute(
                "AllGather", ALU.bypass, replica_groups=groups,
                ins=[ag_in[:]], outs=[ag_out[:]],
            )
            ytf = persist.tile([PT, NCT, T], BF16)
            nc.sync.dma_start(ytf[:], ag_out[:].rearrange("(c p) t -> p c t", p=PT))

            # ---- output projection (column shard) ----
            for ti in range(NT):
                t0 = ti * PT
                pp = psum.tile([PT, QW], F32, tag="qp", bufs=1)
                for ci in range(NCT):
                    nc.tensor.matmul(
                        pp[:], ytf[:, ci, t0:t0 + PT], wp_s[:, ci, :],
                        start=(ci == 0), stop=(ci == NCT - 1),
                    )
                ot = wk.tile([PT, QW], F32, tag="ot", bufs=2)
                nc.vector.tensor_copy(ot[:], pp[:])
                nc.sync.dma_start(out[t0:t0 + PT, :], ot[:])

    nc.compile()
    return nc


def _tables(q_scale, k_scale):
    inv_freq = THETA ** (-np.arange(0, HS, 2, dtype=np.float64) / HS)
    ang = np.arange(T, dtype=np.float64)[:, None] * inv_freq[None, :]
    cosw = np.concatenate([np.cos(ang), np.cos(ang)], 1)  # (T, 128)
    sinw = np.concatenate([np.sin(ang), np.sin(ang)], 1)
    qs = np.asarray(q_scale, np.float64)
    ks = np.asarray(k_scale, np.float64)
    qs_rot = np.concatenate([qs[H2:], qs[:H2]])
    ks_rot = np.concatenate([ks[H2:], ks[:H2]])
    s = 1.0 / math.sqrt(HS)
    cosq = np.tile((cosw * qs[None, :] * s).astype(BF), (1, QH))
    sinq = np.tile((sinw * qs_rot[None, :] * s).astype(BF), (1, QH))
    cosk = (cosw * ks[None, :]).astype(BF)
    sink = (sinw * ks_rot[None, :]).astype(BF)
    return cosq, sinq, cosk, sink


def kernel(x, Wq, Wkv, Wproj, q_scale, k_scale, v1, value_lambda, layer_idx):
    x = np.asarray(x, np.float32)
    Wq = np.asarray(Wq, np.float32)
    Wkv = np.asarray(Wkv, np.float32)
    Wproj = np.asarray(Wproj, np.float32)

    li = int(np.asarray(layer_idx))
    mix = (v1 is not None) and (value_lambda is not None) and li > 0
    lam = float(np.asarray(value_lambda).reshape(())) if mix else 1.0

    cosq, sinq, cosk, sink = _tables(q_scale, k_scale)
    mneg = (np.triu(np.ones((PT, PT), np.float32), k=1) * -1e30).astype(np.float32)

    if "nc" not in _CACHE:
        _CACHE["nc"] = _build()
    nc = _CACHE["nc"]

    in_maps = []
    for core in range(NCORES):
        b, r = core // TP, core % TP
        kcols = Wkv[:, r * HS:(r + 1) * HS]
        vcols = Wkv[:, NKV * HS + r * HS: NKV * HS + (r + 1) * HS]
        if mix:
            v1s_np = ((1.0 - lam) * np.asarray(v1, np.float32)[b, :, r, :]).astype(np.float32)
        else:
            v1s_np = np.zeros((T, HS), np.float32)
        in_maps.append({
            "xT": np.ascontiguousarray(x[b].T).astype(BF),
            "wq": Wq[:, r * QW:(r + 1) * QW].astype(BF),
            "wkv": np.ascontiguousarray(np.concatenate([kcols, vcols], 1)).astype(BF),
            "wp": np.ascontiguousarray(Wproj[:, r * QW:(r + 1) * QW]).astype(BF),
            "v1s": v1s_np,
            "cosq": cosq, "sinq": sinq, "cosk": cosk, "sink": sink,
            "mneg": mneg,
        })

    trace = bool(int(os.environ.get("BASS_KERNEL_TRACE", "0")))
    res = run_bass_kernel_spmd(nc, in_maps, core_ids=list(range(NCORES)), trace=trace)
    _CACHE["last"] = res

    y = np.empty((B, T, C), np.float32)
    for core in range(NCORES):
        b, r = core // TP, core % TP
        y[b, :, r * QW:(r + 1) * QW] = np.asarray(res.results[core]["out"])
    return y


# revision 8
# speedup vs baseline: 1.1994x; 1.1994x over previous
"""GQA attention block (RMSNorm-QK + RoPE + causal attention + proj) on 8 TRN2 cores.

Sharding: DP=2 over batch x TP=4 over heads (4 q heads + 1 kv head per core).
Per core: x[b] @ Wq_shard / Wkv_shard -> q,k,v; RMSNorm+RoPE (cos/sin tables
precomputed on host, q_scale/k_scale and 1/sqrt(HS) baked in); causal
flash-ish attention in bf16 with f32 softmax stats; AllGather of y^T over the
4 TP ranks; column-parallel Wproj. Host pre-transposes x so the device never
transposes activations for the projections.
"""

import math
import os
import sys

import numpy as np

for _p in ("/opt/trn_rl_repo", "/root/.axon_site/_ro/trn_rl_repo"):
    if os.path.isdir(_p) and _p not in sys.path:
        sys.path.insert(0, _p)

import ml_dtypes

import concourse.bacc as bacc
import concourse.mybir as mybir
import concourse.tile as tile
from concourse import masks
from concourse.bass_utils import run_bass_kernel_spmd

BF16 = mybir.dt.bfloat16
F32 = mybir.dt.float32
AX = mybir.AxisListType
ALU = mybir.AluOpType
AF = mybir.ActivationFunctionType

B, T, C = 2, 2048, 2048
NH, NKV, HS = 16, 4, 128
TP = 4                # tensor-parallel ranks per batch element
QH = NH // TP         # q heads per core
QW = QH * HS          # 512
PT = 128
NT = T // PT          # 16
NCT = C // PT         # 16
H2 = HS // 2
EPS = 1e-6
THETA = 10000.0
NCORES = 8
BF = ml_dtypes.bfloat16

_CACHE = {}


def _build(loop_r=None):
    nc = bacc.Bacc(None, target_bir_lowering=False, num_devices=NCORES)

    xT = nc.declare_dram_parameter("xT", [C, T], BF16, isOutput=False)
    wq = nc.declare_dram_parameter("wq", [C, QW], BF16, isOutput=False)
    wkv = nc.declare_dram_parameter("wkv", [C, 2 * HS], BF16, isOutput=False)
    wp = nc.declare_dram_parameter("wp", [C, QW], BF16, isOutput=False)
    v1s = nc.declare_dram_parameter("v1s", [T, HS], F32, isOutput=False)
    cosq = nc.declare_dram_parameter("cosq", [T, QW], BF16, isOutput=False)
    sinq = nc.declare_dram_parameter("sinq", [T, QW], BF16, isOutput=False)
    cosk = nc.declare_dram_parameter("cosk", [T, HS], BF16, isOutput=False)
    sink = nc.declare_dram_parameter("sink", [T, HS], BF16, isOutput=False)
    mneg = nc.declare_dram_parameter("mneg", [PT, PT], F32, isOutput=False)
    out = nc.declare_dram_parameter("out", [T, QW], F32, isOutput=True)

    groups = [[0, 1, 2, 3], [4, 5, 6, 7]]

    with tile.TileContext(nc) as tc:
        with (
            tc.tile_pool(name="const", bufs=1) as const,
            tc.tile_pool(name="persist", bufs=1) as persist,
            tc.tile_pool(name="psum", bufs=1, space="PSUM") as psum,
            tc.tile_pool(name="wk", bufs=3) as wk,
            tc.tile_pool(name="dram", bufs=1, space="DRAM") as dram,
        ):
            ident = const.tile([PT, PT], BF16)
            masks.make_identity(nc, ident[:])
            maskt = const.tile([PT, PT], F32)
            nc.sync.dma_start(maskt[:], mneg[:])
            eps_t = const.tile([PT, 1], F32)
            nc.gpsimd.memset(eps_t[:], EPS)

            wq_s = persist.tile([PT, NCT, QW], BF16)
            nc.sync.dma_start(wq_s[:], wq[:].rearrange("(c p) m -> p c m", p=PT))
            wkv_s = persist.tile([PT, NCT, 2 * HS], BF16)
            nc.sync.dma_start(wkv_s[:], wkv[:].rearrange("(c p) m -> p c m", p=PT))
            wp_s = persist.tile([PT, NCT, QW], BF16)
            nc.sync.dma_start(wp_s[:], wp[:].rearrange("(c p) m -> p c m", p=PT))

            qT_s = persist.tile([PT, QH, T], BF16)
            kT_s = persist.tile([PT, T], BF16)
            v_s = persist.tile([PT, NT, HS], BF16)
            yT_s = persist.tile([PT, QH, T], BF16)

            ag_in = dram.tile([QW, T], BF16)
            ag_out = dram.tile([C, T], BF16)

            def _load_xt(xt_s):
                nc.sync.dma_start(xt_s[:], xT[:].rearrange("(c p) t -> p c t", p=PT))

            def _stage23(xt_s):
                for ti in range(NT):
                    t0 = ti * PT
                    # ---- QKV projection (lhsT = xT tile, rhs = weight) ----
                    qp = psum.tile([PT, QW], F32, tag="qp", bufs=1)
                    for ci in range(NCT):
                        nc.tensor.matmul(
                            qp[:], xt_s[:, ci, t0:t0 + PT], wq_s[:, ci, :],
                            start=(ci == 0), stop=(ci == NCT - 1),
                        )
                    kvp = psum.tile([PT, 2 * HS], F32, tag="kvp", bufs=1)
                    for ci in range(NCT):
                        nc.tensor.matmul(
                            kvp[:], xt_s[:, ci, t0:t0 + PT], wkv_s[:, ci, :],
                            start=(ci == 0), stop=(ci == NCT - 1),
                        )

                    # ---- RMSNorm stats ----
                    sq = wk.tile([PT, QW], F32, tag="sq")
                    nc.scalar.square(sq[:], qp[:])
                    ms = wk.tile([PT, QH], F32, tag="ms")
                    nc.vector.tensor_reduce(
                        ms[:], sq[:].rearrange("p (h d) -> p h d", d=HS), AX.X, ALU.add
                    )
                    sqk = wk.tile([PT, HS], F32, tag="sqk")
                    nc.scalar.square(sqk[:], kvp[:, 0:HS])
                    msk = wk.tile([PT, 1], F32, tag="msk")
                    nc.vector.tensor_reduce(msk[:], sqk[:], AX.X, ALU.add)

                    rs = wk.tile([PT, QH], F32, tag="rs")
                    nc.scalar.activation(rs[:], ms[:], AF.Sqrt, bias=eps_t[:], scale=1.0 / HS)
                    nc.vector.reciprocal(rs[:], rs[:])
                    rsk = wk.tile([PT, 1], F32, tag="rsk")
                    nc.scalar.activation(rsk[:], msk[:], AF.Sqrt, bias=eps_t[:], scale=1.0 / HS)
                    nc.vector.reciprocal(rsk[:], rsk[:])

                    # ---- normalize (per-head scalar) ----
                    qn = wk.tile([PT, QH, HS], BF16, tag="qn")
                    for h in range(QH):
                        nc.vector.tensor_scalar_mul(
                            qn[:, h, :], qp[:, h * HS:(h + 1) * HS], rs[:, h:h + 1]
                        )
                    kn = wk.tile([PT, HS], BF16, tag="kn")
                    nc.vector.tensor_scalar_mul(kn[:], kvp[:, 0:HS], rsk[:])

                    # ---- RoPE: out = z*cos + rot(z)*sin, tables baked with scales ----
                    qrot = wk.tile([PT, QH, HS], BF16, tag="qrot")
                    nc.vector.tensor_scalar_mul(qrot[:, :, 0:H2], qn[:, :, H2:HS], -1.0)
                    nc.vector.tensor_copy(qrot[:, :, H2:HS], qn[:, :, 0:H2])
                    krot = wk.tile([PT, HS], BF16, tag="krot")
                    nc.vector.tensor_scalar_mul(krot[:, 0:H2], kn[:, H2:HS], -1.0)
                    nc.vector.tensor_copy(krot[:, H2:HS], kn[:, 0:H2])

                    cqt = wk.tile([PT, QW], BF16, tag="cqt")
                    nc.sync.dma_start(cqt[:], cosq[t0:t0 + PT, :])
                    sqt = wk.tile([PT, QW], BF16, tag="sqt")
                    nc.sync.dma_start(sqt[:], sinq[t0:t0 + PT, :])
                    ckt = wk.tile([PT, HS], BF16, tag="ckt")
                    nc.sync.dma_start(ckt[:], cosk[t0:t0 + PT, :])
                    skt = wk.tile([PT, HS], BF16, tag="skt")
                    nc.sync.dma_start(skt[:], sink[t0:t0 + PT, :])

                    qn2 = qn[:].rearrange("p h d -> p (h d)")
                    qrot2 = qrot[:].rearrange("p h d -> p (h d)")
                    qr = wk.tile([PT, QW], BF16, tag="qr")
                    nc.vector.tensor_tensor(qr[:], qn2, cqt[:], ALU.mult)
                    nc.vector.tensor_tensor(qrot2, qrot2, sqt[:], ALU.mult)
                    nc.vector.tensor_tensor(qr[:], qr[:], qrot2, ALU.add)

                    kr = wk.tile([PT, HS], BF16, tag="kr")
                    nc.vector.tensor_tensor(kr[:], kn[:], ckt[:], ALU.mult)
                    nc.vector.tensor_tensor(krot[:], krot[:], skt[:], ALU.mult)
                    nc.vector.tensor_tensor(kr[:], kr[:], krot[:], ALU.add)

                    # ---- v mix ----
                    v1t = wk.tile([PT, HS], F32, tag="v1t")
                    nc.sync.dma_start(v1t[:], v1s[t0:t0 + PT, :])
                    nc.vector.tensor_tensor(v_s[:, ti, :], kvp[:, HS:2 * HS], v1t[:], ALU.add)

                    # ---- transposes q,k -> qT, kT ----
                    qr3 = qr[:].rearrange("p (h d) -> p h d", d=HS)
                    for h in range(QH):
                        tq = psum.tile([PT, PT], BF16, tag="tq", bufs=2)
                        nc.tensor.transpose(tq[:], qr3[:, h, :], ident[:])
                        nc.vector.tensor_copy(qT_s[:, h, t0:t0 + PT], tq[:])
                    tk = psum.tile([PT, PT], BF16, tag="tq", bufs=2)
                    nc.tensor.transpose(tk[:], kr[:], ident[:])
                    nc.vector.tensor_copy(kT_s[:, t0:t0 + PT], tk[:])

                    # ---- causal attention row ti ----
                    nk = ti + 1
                    nchunk = (nk + 3) // 4
                    for h in range(QH):
                        prow = wk.tile([PT, T], BF16, tag="prow", bufs=2)
                        rsp = wk.tile([PT, 4], F32, tag="rsp", bufs=2)
                        for ch in range(nchunk):
                            c0 = ch * 512
                            cw = min(512, nk * PT - c0)
                            sp = psum.tile([PT, 512], F32, tag="sp", bufs=2)
                            nc.tensor.matmul(
                                sp[:, 0:cw], qT_s[:, h, t0:t0 + PT], kT_s[:, c0:c0 + cw],
                                start=True, stop=True,
                            )
                            if ch == nchunk - 1:
                                dc = cw - PT
                                nc.vector.tensor_tensor(
                                    sp[:, dc:dc + PT], sp[:, dc:dc + PT], maskt[:], ALU.add
                                )
                            nc.scalar.activation(
                                prow[:, c0:c0 + cw], sp[:, 0:cw], AF.Exp,
                                accum_out=rsp[:, ch:ch + 1],
                            )
                        rinv = wk.tile([PT, 1], F32, tag="rinv", bufs=2)
                        if nchunk > 1:
                            rsum = wk.tile([PT, 1], F32, tag="rsum", bufs=2)
                            nc.vector.tensor_reduce(rsum[:], rsp[:, 0:nchunk], AX.X, ALU.add)
                            nc.vector.reciprocal(rinv[:], rsum[:])
                        else:
                            nc.vector.reciprocal(rinv[:], rsp[:, 0:1])
                        nc.vector.tensor_scalar_mul(prow[:, 0:nk * PT], prow[:, 0:nk * PT], rinv[:])

                        yp = psum.tile([PT, HS], F32, tag="yp", bufs=2)
                        for j in range(nk):
                            ptp = psum.tile([PT, PT], BF16, tag="tq", bufs=2)
                            nc.tensor.transpose(ptp[:], prow[:, j * PT:(j + 1) * PT], ident[:])
                            pts = wk.tile([PT, PT], BF16, tag="pts", bufs=3)
                            nc.vector.tensor_copy(pts[:], ptp[:])
                            nc.tensor.matmul(
                                yp[:], v_s[:, j, :], pts[:],
                                start=(j == 0), stop=(j == nk - 1),
                            )
                        nc.vector.tensor_copy(yT_s[:, h, t0:t0 + PT], yp[:])

            def _proj(ytf):
                # ---- output projection (column shard) ----
                for ti in range(NT):
                    t0 = ti * PT
                    pp = psum.tile([PT, QW], F32, tag="qp", bufs=1)
                    for ci in range(NCT):
                        nc.tensor.matmul(
                            pp[:], ytf[:, ci, t0:t0 + PT], wp_s[:, ci, :],
                            start=(ci == 0), stop=(ci == NCT - 1),
                        )
                    ot = wk.tile([PT, QW], F32, tag="ot", bufs=2)
                    nc.vector.tensor_copy(ot[:], pp[:])
                    nc.sync.dma_start(out[t0:t0 + PT, :], ot[:])

            if loop_r is None:
                with tc.tile_pool(name="xtp", bufs=1) as xtp:
                    xt_s = xtp.tile([PT, NCT, T], BF16)
                    _load_xt(xt_s)
                    _stage23(xt_s)
                # ---- AllGather y^T over the TP group ----
                nc.sync.dma_start(ag_in[:].rearrange("(h p) t -> p h t", p=PT), yT_s[:])
                nc.gpsimd.collective_compute(
                    "AllGather", ALU.bypass, replica_groups=groups,
                    ins=[ag_in[:]], outs=[ag_out[:]],
                )
                with tc.tile_pool(name="ytfp", bufs=1) as ytfp:
                    ytf = ytfp.tile([PT, NCT, T], BF16)
                    nc.sync.dma_start(ytf[:], ag_out[:].rearrange("(c p) t -> p c t", p=PT))
                    _proj(ytf)
            else:
                # timing-only build: loop the whole compute body on-device;
                # proj consumes xt_s (same shape as gathered y^T) - numerics
                # are wrong but per-iteration work matches the real kernel
                # minus the AllGather.
                with tc.tile_pool(name="xtp", bufs=1) as xtp:
                    xt_s = xtp.tile([PT, NCT, T], BF16)
                    with tc.For_i(0, loop_r, 1):
                        _load_xt(xt_s)
                        _stage23(xt_s)
                        _proj(xt_s)

    nc.compile()
    return nc


def _tables(q_scale, k_scale):
    inv_freq = THETA ** (-np.arange(0, HS, 2, dtype=np.float64) / HS)
    ang = np.arange(T, dtype=np.float64)[:, None] * inv_freq[None, :]
    cosw = np.concatenate([np.cos(ang), np.cos(ang)], 1)  # (T, 128)
    sinw = np.concatenate([np.sin(ang), np.sin(ang)], 1)
    qs = np.asarray(q_scale, np.float64)
    ks = np.asarray(k_scale, np.float64)
    qs_rot = np.concatenate([qs[H2:], qs[:H2]])
    ks_rot = np.concatenate([ks[H2:], ks[:H2]])
    s = 1.0 / math.sqrt(HS)
    cosq = np.tile((cosw * qs[None, :] * s).astype(BF), (1, QH))
    sinq = np.tile((sinw * qs_rot[None, :] * s).astype(BF), (1, QH))
    cosk = (cosw * ks[None, :]).astype(BF)
    sink = (sinw * ks_rot[None, :]).astype(BF)
    return cosq, sinq, cosk, sink


def _make_in_maps(x, Wq, Wkv, Wproj, q_scale, k_scale, v1, value_lambda, layer_idx):
    x = np.asarray(x, np.float32)
    Wq = np.asarray(Wq, np.float32)
    Wkv = np.asarray(Wkv, np.float32)
    Wproj = np.asarray(Wproj, np.float32)

    li = int(np.asarray(layer_idx))
    mix = (v1 is not None) and (value_lambda is not None) and li > 0
    lam = float(np.asarray(value_lambda).reshape(())) if mix else 1.0

    cosq, sinq, cosk, sink = _tables(q_scale, k_scale)
    mneg = (np.triu(np.ones((PT, PT), np.float32), k=1) * -1e30).astype(np.float32)

    in_maps = []
    for core in range(NCORES):
        b, r = core // TP, core % TP
        kcols = Wkv[:, r * HS:(r + 1) * HS]
        vcols = Wkv[:, NKV * HS + r * HS: NKV * HS + (r + 1) * HS]
        if mix:
            v1s_np = ((1.0 - lam) * np.asarray(v1, np.float32)[b, :, r, :]).astype(np.float32)
        else:
            v1s_np = np.zeros((T, HS), np.float32)
        in_maps.append({
            "xT": np.ascontiguousarray(x[b].T).astype(BF),
            "wq": Wq[:, r * QW:(r + 1) * QW].astype(BF),
            "wkv": np.ascontiguousarray(np.concatenate([kcols, vcols], 1)).astype(BF),
            "wp": np.ascontiguousarray(Wproj[:, r * QW:(r + 1) * QW]).astype(BF),
            "v1s": v1s_np,
            "cosq": cosq, "sinq": sinq, "cosk": cosk, "sink": sink,
            "mneg": mneg,
        })
    return in_maps


def kernel(x, Wq, Wkv, Wproj, q_scale, k_scale, v1, value_lambda, layer_idx):
    in_maps = _make_in_maps(x, Wq, Wkv, Wproj, q_scale, k_scale, v1,
                            value_lambda, layer_idx)
    if "nc" not in _CACHE:
        _CACHE["nc"] = _build()
    nc = _CACHE["nc"]

    trace = bool(int(os.environ.get("BASS_KERNEL_TRACE", "0")))
    res = run_bass_kernel_spmd(nc, in_maps, core_ids=list(range(NCORES)), trace=trace)
    _CACHE["last"] = res

    y = np.empty((B, T, C), np.float32)
    for core in range(NCORES):
        b, r = core // TP, core % TP
        y[b, :, r * QW:(r + 1) * QW] = np.asarray(res.results[core]["out"])
    return y
